# revision 3
# baseline (speedup 1.0000x reference)
"""Fused 8-layer transformer for TRN2, data-parallel over batch across 8
NeuronCores — wave-pipelined v2.

Keeps the baseline's feature-major layout (xT[feature 128-part, hc, token])
but restructures the per-layer schedule into 16 half-token stages
(s = 2*layer + half, half = 512 tokens = 2 local batches). Attention of
stage s runs on ACT/DVE while the dense block of stage s-1 (out-proj, LN2,
FFN, next-layer LN1+QKV for stage s+1) keeps the PE busy — the emission
interleaves the two streams with explicit fill hints so attention matmul
bursts land exactly where the PE would otherwise stall (LN chain joins),
keeping it busy enough that the HAM clock gate never throttles.

vs baseline:
- fp8(e4m3) DoubleRow matmuls for QKV/out-proj/ctx (2x PE throughput);
  FFN stays bf16 (fp8 FFN measured 2.6e-2 rel err on host sim - over gate).
- attn_bias is loaded raw (fp8, resident in SBUF) and pre-accumulated into
  the scores PSUM via an identity matmul, so softmax is one ACT Exp
  (bias -2.5 guards fp8 overflow; the shift cancels in normalization).
- LN runs in 256-token quarters: the stats->rstd chain for quarter 0
  overlaps quarter 1's work, halving the cross-engine latency window.
"""

import numpy as np
import ml_dtypes
from contextlib import ExitStack

import bass_rust
import concourse.bass as bass
import concourse.tile as tile
from concourse import mybir
from concourse.bass_utils import run_bass_kernel_spmd

BF16 = ml_dtypes.bfloat16
E4M3 = ml_dtypes.float8_e4m3

B, N, H, HEADS, DH, F, L = 32, 256, 512, 8, 64, 2048, 8
NC = 8
BL = B // NC            # local batch = 4
T = BL * N              # local tokens = 1024
TH = T // 2             # tokens per stage (half) = 512
S = 2 * L               # 16 stages
EPS = 1e-5
EXP_SHIFT = -2.5        # exp(s+b-2.5): keeps fp8 probs < 240; cancels in norm

FP32 = mybir.dt.float32
BF = mybir.dt.bfloat16
F8 = mybir.dt.float8e4
AF = mybir.ActivationFunctionType
OP = mybir.AluOpType
DR = mybir.MatmulPerfMode.DoubleRow


def _legalize_sync(nc):
    # walrus codegen encodes at most 1 sem wait + 1 sem update per
    # instruction; hoist excess onto same-engine NoOps.
    uid = 0
    for fn in nc.m.functions:
        for blk in fn.blocks:
            out = []
            changed = False
            for ins in blk.instructions:
                si = ins.sync_info
                if si is not None and (len(si.on_wait) > 1 or len(si.on_update) > 1):
                    waits = list(si.on_wait)
                    upds = list(si.on_update)
                    for w in waits[:-1]:
                        uid += 1
                        nop = bass_rust.InstNoOp(name=f"LGLW-{uid}", engine=ins.engine)
                        nop.sync_info = mybir.SyncInfo(on_wait=[w], on_update=[])
                        out.append(nop)
                    post = []
                    if len(upds) > 1:
                        opname = type(ins).__name__
                        assert "DMA" not in opname and "Dma" not in opname, ins.name
                        for u in upds[1:]:
                            uid += 1
                            nop = bass_rust.InstNoOp(
                                name=f"LGLU-{uid}", engine=ins.engine)
                            nop.sync_info = mybir.SyncInfo(on_wait=[], on_update=[u])
                            post.append(nop)
                        upds = upds[:1]
                    ins.sync_info = mybir.SyncInfo(on_wait=waits[-1:], on_update=upds)
                    out.append(ins)
                    out.extend(post)
                    changed = True
                else:
                    out.append(ins)
            if changed:
                blk.instructions = out


def _build_nc():
    nc = bass.Bass("TRN2", target_bir_lowering=False, debug=False)

    def din(name, shape, dt):
        return nc.dram_tensor(name, shape, dt, kind="ExternalInput").ap()

    x_in = din("x_in", [128, 4, T], FP32)
    eb_in = din("eb_in", [128, BL * HEADS, 512], F8)
    wq_in = din("wq_in", [L, 128, 2048], F8)   # [p, i2, c2, oc4, 128]
    wk_in = din("wk_in", [L, 128, 2048], F8)
    wv_in = din("wv_in", [L, 128, 2048], F8)   # [p, i2, c2, 512]
    wo_in = din("wo_in", [L, 128, 2048], F8)   # [p, i2, c2, oc4, 128]
    w1_in = din("w1_in", [L, 128, 8192], BF)   # [p, hc4, fc16, 128]
    w2_in = din("w2_in", [L, 128, 8192], BF)   # [p, fc16, oc4, 128]
    bq_in = din("bq_in", [L, 128, 4], FP32)
    bk_in = din("bk_in", [L, 128, 4], FP32)
    b1_in = din("b1_in", [L, 128, 16], FP32)
    bo_in = din("bo_in", [L, 128, 4], FP32)
    b2_in = din("b2_in", [L, 128, 4], FP32)
    sel_in = din("sel_in", [64, 8, 128], BF)
    id_in = din("id_in", [128, 128], F8)
    y_out = nc.dram_tensor("y_out", [128, 4, T], FP32, kind="ExternalOutput").ap()

    with ExitStack() as stk:
        tc = stk.enter_context(tile.TileContext(nc))
        const = stk.enter_context(tc.tile_pool(name="const", bufs=1))
        wts = stk.enter_context(tc.tile_pool(name="wts", bufs=2))
        work = stk.enter_context(tc.tile_pool(name="work", bufs=2))
        smalls = stk.enter_context(tc.tile_pool(name="smalls", bufs=2))
        pp = stk.enter_context(tc.tile_pool(name="pp", bufs=1, space="PSUM"))

        xT = const.tile([128, 4, T], FP32, tag="xT")
        eb_res = const.tile([128, BL * HEADS, 512], F8, tag="eb_res")
        sel_t = const.tile([64, 8, 128], BF, tag="sel_t")
        ident = const.tile([128, 128], F8, tag="ident")
        ones = const.tile([128, 1], BF, tag="ones")
        nc.vector.memset(ones, 1.0 / H)
        ones_r = const.tile([1, 128], BF, tag="ones_r")
        nc.vector.memset(ones_r, 1.0)
        eps_t = const.tile([1, 1], FP32, tag="eps")
        nc.vector.memset(eps_t, EPS)
        eshift = const.tile([128, 1], FP32, tag="eshift")
        nc.vector.memset(eshift, EXP_SHIFT)
        # per-parity v tiles: [k-part, (b2, ktc2), head, d|ones|pad]
        # (last dim padded to 72: dual-fp8 LDWEIGHTS needs pair stride %16==0)
        v_aug = []
        for par in range(2):
            va = const.tile([128, 4, HEADS, 72], F8, tag=f"v_aug{par}",
                            name=f"v_aug{par}")
            nc.vector.memset(va[:, :, :, 64:65], 1.0)
            v_aug.append(va)

        nc.sync.dma_start(xT, x_in)
        nc.sync.dma_start(eb_res, eb_in)
        nc.sync.dma_start(sel_t, sel_in)
        nc.sync.dma_start(ident, id_in)

        wt = {}     # layer -> weight tile handles
        stg = {}    # stage -> activation tile handles

        def load_qkv(l):
            wq_t = wts.tile([128, 2, 2, 4, 128], F8, tag="wq", name="wq_t")
            wk_t = wts.tile([128, 2, 2, 4, 128], F8, tag="wk", name="wk_t")
            wv_t = wts.tile([128, 2, 2, 512], F8, tag="wv", name="wv_t")
            bq_t = smalls.tile([128, 4], FP32, tag="bq", name="bq_t")
            bk_t = smalls.tile([128, 4], FP32, tag="bk", name="bk_t")
            nc.sync.dma_start(wq_t, wq_in[l].rearrange("p (i c oc o) -> p i c oc o", i=2, c=2, oc=4))
            nc.sync.dma_start(wk_t, wk_in[l].rearrange("p (i c oc o) -> p i c oc o", i=2, c=2, oc=4))
            nc.sync.dma_start(wv_t, wv_in[l].rearrange("p (i c f) -> p i c f", i=2, c=2))
            nc.sync.dma_start(bq_t, bq_in[l])
            nc.sync.dma_start(bk_t, bk_in[l])
            wt.setdefault(l, {}).update(wq=wq_t, wk=wk_t, wv=wv_t, bq=bq_t, bk=bk_t)

        def load_ffn(l):
            wo_t = wts.tile([128, 2, 2, 4, 128], F8, tag="wo", name="wo_t")
            w1_t = wts.tile([128, 8192], BF, tag="w1", name="w1_t")
            w2_t = wts.tile([128, 8192], BF, tag="w2", name="w2_t")
            b1_t = smalls.tile([128, 16], FP32, tag="b1", name="b1_t")
            bo_t = smalls.tile([128, 4], FP32, tag="bo", name="bo_t")
            b2_t = smalls.tile([128, 4], FP32, tag="b2", name="b2_t")
            nc.sync.dma_start(wo_t, wo_in[l].rearrange("p (i c oc o) -> p i c oc o", i=2, c=2, oc=4))
            nc.sync.dma_start(w1_t, w1_in[l])
            nc.sync.dma_start(w2_t, w2_in[l])
            nc.sync.dma_start(b1_t, b1_in[l])
            nc.sync.dma_start(bo_t, bo_in[l])
            nc.sync.dma_start(b2_t, b2_in[l])
            wt.setdefault(l, {}).update(wo=wo_t, w1=w1_t, w2=w2_t, b1=b1_t,
                                        bo=bo_t, b2=b2_t)

        def stats_tiles():
            # one accumulation group per bank: [0,0,:]=x-sums, [0,1,:]=sq-sums
            st0 = pp.tile([1, 2, 256], FP32, tag="lnps", name="st0", bufs=2)
            st1 = pp.tile([1, 2, 256], FP32, tag="lnps", name="st1", bufs=2)
            return st0, st1

        def stage_xsq(xs):
            # xsq[:, qu, 0, :]=bf16(x), xsq[:, qu, 1, :]=bf16(x^2) so each
            # quarter's stats matmul is a single [128, (2,256)] moving operand
            xsq = work.tile([128, 2, 2, 256], BF, tag="xsq", name="xsq", bufs=4)
            xr = xs.rearrange("p (u t) -> p u t", u=2)
            nc.vector.tensor_scalar_add(xsq[:, :, 0, :], xr, 0.0)
            nc.vector.tensor_mul(xsq[:, :, 1, :], xr, xr)
            return xsq

        def stats_mm(st, xsq, qu, hc):
            nc.tensor.matmul(st[0:1, :, :], ones, xsq[:, qu, :, :],
                             start=hc == 0, stop=hc == 3)

        def ln_chain_q(st, nm):
            # st[0,0]=mean, st[0,1]=E[x^2] over the quarter's tokens
            msq = smalls.tile([1, 256], FP32, tag="msq", name=f"msq{nm}")
            nc.scalar.activation(msq, st[0:1, 0, :], AF.Square)
            var = smalls.tile([1, 256], FP32, tag="var", name=f"var{nm}")
            nc.vector.tensor_sub(var, st[0:1, 1, :], msq)
            lnv = smalls.tile([1, 256], FP32, tag="lnv", name=f"lnv{nm}")
            nc.scalar.activation(lnv, var, AF.Ln, bias=eps_t)
            rstd = smalls.tile([1, 256], BF, tag="rstd", name=f"rstd{nm}")
            nc.scalar.activation(rstd, lnv, AF.Exp, scale=-0.5)
            mur = smalls.tile([1, 256], BF, tag="mur", name=f"mur{nm}")
            with nc.allow_low_precision(reason="bf16 bc-matmul operands"):
                nc.vector.tensor_mul(mur, st[0:1, 0, :], rstd)
            return rstd, mur

        def ln_finish(st0, st1, xsqs, nm, ytile, to, low_prec_reason):
            """Quarter-0 chain overlaps quarter-1 stats; yields A-fill hints."""
            rm = [ln_chain_q(st0, nm + "a")]
            yield 1
            for hc in range(4):
                stats_mm(st1, xsqs[hc], 1, hc)
            rm.append(ln_chain_q(st1, nm + "b"))
            yield 2
            for qu in range(2):
                rstd, mur = rm[qu]
                qsl = slice(qu * 256, (qu + 1) * 256)
                bc = pp.tile([128, 2, 256], FP32, tag="lnps", name="bc" + nm, bufs=2)
                nc.tensor.matmul(bc[:, 0, :], ones_r, rstd, start=True, stop=True)
                nc.tensor.matmul(bc[:, 1, :], ones_r, mur, start=True, stop=True)
                bcs = smalls.tile([128, 2, 256], BF, tag="bcs",
                                  name="bcs" + nm, bufs=2)
                with nc.allow_low_precision(reason="bf16 bc"):
                    nc.scalar.activation(bcs, bc, AF.Copy)
                for hc in range(4):
                    tmp = work.tile([128, 256], BF, tag="lntmp",
                                    name="lntmp" + nm, bufs=2)
                    with nc.allow_low_precision(reason=low_prec_reason):
                        nc.vector.tensor_mul(tmp, xsqs[hc][:, qu, 0, :],
                                             bcs[:, 0, :])
                        nc.vector.tensor_sub(ytile[:, hc, qsl], tmp,
                                             bcs[:, 1, :])
                yield 1 if qu == 0 else 2

        def emit_ln1(ns):
            """LN1 for stage ns -> y1 (yields A-fill hints)."""
            to = (ns % 2) * TH
            st0, st1 = stats_tiles()
            xsqs = []
            for hc in range(4):
                xs = xT[:, hc, to:to + TH]
                xsqs.append(stage_xsq(xs))
                stats_mm(st0, xsqs[hc], 0, hc)
                yield 0
            y1 = work.tile([128, 4, TH], F8, tag="y1", name="y1", bufs=2)
            yield from ln_finish(st0, st1, xsqs, f"l1_{ns}", y1, to, "fp8 y1")
            stg[ns] = dict(y1=y1)

        def emit_qkv(ns):
            """QKV + V matmuls for stage ns (emitted as one block)."""
            l = ns // 2
            w = wt[l]
            y1 = stg[ns]["y1"]
            qT = work.tile([128, 4, TH], F8, tag="qT", name="qT", bufs=2)
            kT = work.tile([128, 4, TH], F8, tag="kT", name="kT", bufs=2)
            for oc in range(4):
                pq = pp.tile([128, TH], FP32, tag="pd", name="pq", bufs=3)
                for c in range(2):
                    nc.tensor.matmul(pq, w["wq"][:, :, c, oc, :], y1[:, 2 * c:2 * c + 2, :],
                                     start=c == 0, stop=c == 1, perf_mode=DR)
                with nc.allow_low_precision(reason="fp8 q"):
                    nc.vector.tensor_scalar_add(qT[:, oc, :], pq, w["bq"][:, oc:oc + 1])
                pk = pp.tile([128, TH], FP32, tag="pd", name="pk", bufs=3)
                for c in range(2):
                    nc.tensor.matmul(pk, w["wk"][:, :, c, oc, :], y1[:, 2 * c:2 * c + 2, :],
                                     start=c == 0, stop=c == 1, perf_mode=DR)
                with nc.allow_low_precision(reason="fp8 k"):
                    nc.scalar.activation(kT[:, oc, :], pk, AF.Identity,
                                         bias=w["bk"][:, oc:oc + 1])
            va = v_aug[ns % 2]
            for tcc in range(4):
                pv = pp.tile([128, TH], FP32, tag="pd", name="pv", bufs=3)
                for c in range(2):
                    nc.tensor.matmul(pv, y1[:, 2 * c:2 * c + 2, tcc * 128:(tcc + 1) * 128],
                                     w["wv"][:, :, c, :],
                                     start=c == 0, stop=c == 1, perf_mode=DR)
                with nc.allow_low_precision(reason="fp8 v"):
                    nc.scalar.activation(
                        va[:, tcc, :, 0:64],
                        pv.rearrange("p (h d) -> p h d", h=HEADS), AF.Copy)
            stg[ns].update(qT=qT, kT=kT)

        def emit_D(i):
            """WO(i) (covers prev normalize window), QKV(i+1), LN2+FFN(i),
            then LN1(i+2). Yields -1 once QKV is emitted (opens A fills)."""
            if i >= 0:
                s = i
                l = s // 2
                to = (s % 2) * TH
                w = wt[l]
                ctx8 = stg[s].pop("ctx8")
                st0, st1 = stats_tiles()
                xsqs = []
                for oc in range(4):
                    po = pp.tile([128, TH], FP32, tag="pd", name="po", bufs=3)
                    for c in range(2):
                        nc.tensor.matmul(po, w["wo"][:, :, c, oc, :],
                                         ctx8[:, 2 * c:2 * c + 2, :],
                                         start=c == 0, stop=c == 1, perf_mode=DR)
                    xs = xT[:, oc, to:to + TH]
                    nc.vector.scalar_tensor_tensor(xs, po, w["bo"][:, oc:oc + 1],
                                                   xs, OP.add, OP.add)
                    xsqs.append(stage_xsq(xs))
                    stats_mm(st0, xsqs[oc], 0, oc)
            if i + 1 <= S - 1:
                emit_qkv(i + 1)
            yield -1
            yield 1
            if i >= 0:
                y2 = work.tile([128, 4, TH], BF, tag="y2", name="y2", bufs=2)
                yield from ln_finish(st0, st1, xsqs, f"l2_{s}", y2, to, "bf16 y2")
                # --- FFN1 ---
                g_all = work.tile([128, 16, TH], BF, tag="g", name="g_all", bufs=1)
                for fc in range(16):
                    pf = pp.tile([128, TH], FP32, tag="pd", name="pf", bufs=3)
                    for hc in range(4):
                        wsl = slice((hc * 16 + fc) * 128, (hc * 16 + fc + 1) * 128)
                        nc.tensor.matmul(pf, w["w1"][:, wsl], y2[:, hc, :],
                                         start=hc == 0, stop=hc == 3)
                    nc.scalar.activation(g_all[:, fc, :], pf, AF.Gelu,
                                         bias=w["b1"][:, fc:fc + 1])
                    if fc % 2 == 1:
                        yield 0
                # --- FFN2 + residual ---
                for oc in range(4):
                    acc = pp.tile([128, TH], FP32, tag="pd", name="acc", bufs=3)
                    for fc in range(16):
                        wsl = slice((fc * 4 + oc) * 128, (fc * 4 + oc + 1) * 128)
                        nc.tensor.matmul(acc, w["w2"][:, wsl], g_all[:, fc, :],
                                         start=fc == 0, stop=fc == 15)
                        if fc == 7:
                            yield 0
                    xs = xT[:, oc, to:to + TH]
                    nc.vector.scalar_tensor_tensor(xs, acc, w["b2"][:, oc:oc + 1],
                                                   xs, OP.add, OP.add)
                    yield 1
            # --- LN1 for stage i+2 ---
            if i + 2 <= S - 1:
                yield from emit_ln1(i + 2)

        def emit_A(i):
            """Attention for stage i: 16 (batch,head) pair steps + norm tail."""
            s = i
            par = s % 2
            qT, kT = stg[s]["qT"], stg[s]["kT"]
            va = v_aug[par]
            ctxT = work.tile([128, 4, TH], BF, tag="ctxT", name="ctxT", bufs=2)
            ctx8 = work.tile([128, 4, TH], F8, tag="ctx8", name="ctx8", bufs=2)
            den = work.tile([64, 256], FP32, tag="den", name="den", bufs=2)
            stg[s]["ctx8"] = ctx8
            pc_cur = [None]

            def emit_ctx(pv_):
                probs, b, hd, p = pv_
                if p % 2 == 0:
                    pc_cur[0] = pp.tile([128, 2, 256], FP32, tag="pc",
                                        name="pc", bufs=1)
                pc = pc_cur[0]
                j = p % 2
                nc.tensor.matmul(pc[0:65, j, :], va[:, 2 * b:2 * b + 2, hd, 0:65],
                                 probs.rearrange("p (t q) -> p t q", t=2),
                                 start=True, stop=True, perf_mode=DR)
                hp = (hd % 2) * 64
                hcq = hd // 2
                nc.vector.tensor_scalar_add(
                    ctxT[hp:hp + 64, hcq, b * 256:(b + 1) * 256],
                    pc[0:64, j, :], 0.0)
                dstg = smalls.tile([1, 256], FP32, tag="dstg", name="dstg",
                                   bufs=4)
                nc.scalar.activation(dstg, pc[64:65, j, :], AF.Copy)
                drow = (p // 8) * 32 + p % 8
                nc.sync.dma_start(den[drow:drow + 1, :], dstg)

            rden = work.tile([64, 256], BF, name="rden", tag="rden", bufs=2)

            def tail(b):
                # batch b's 8 den rows are final: normalize its ctx columns
                r0 = b * 32
                with nc.allow_low_precision(reason="bf16 rden"):
                    nc.vector.reciprocal(rden[r0:r0 + 8, :], den[r0:r0 + 8, :])
                yield
                for hcq in range(4):
                    nbc = pp.tile([128, 256], FP32, tag="pd", name="nbc", bufs=3)
                    nc.tensor.matmul(nbc, sel_t[r0:r0 + 8, b * 4 + hcq, :],
                                     rden[r0:r0 + 8, :], start=True, stop=True)
                    qsl = slice(b * 256, (b + 1) * 256)
                    with nc.allow_low_precision(reason="fp8 ctx"):
                        nc.vector.tensor_mul(ctx8[:, hcq, qsl],
                                             ctxT[:, hcq, qsl], nbc)
                    if hcq % 2 == 1:
                        yield

            pend = []
            for p in range(16):
                b = p // 8
                hd = p % 8
                jj = (2 * par + b) * HEADS + hd
                hp = (hd % 2) * 64
                hcq = hd // 2
                ps = pp.tile([128, 512], FP32, tag="ps", name="ps", bufs=2)
                nc.tensor.matmul(ps, ident, eb_res[:, jj, :], start=True,
                                 stop=False, skip_group_check=True)
                for ktc in range(2):
                    nc.tensor.matmul(
                        ps[:, ktc * 256:(ktc + 1) * 256],
                        kT[hp:hp + 64, hcq, b * 256 + ktc * 128:b * 256 + (ktc + 1) * 128],
                        qT[hp:hp + 64, hcq, b * 256:(b + 1) * 256],
                        start=False, stop=ktc == 1, skip_group_check=True)
                probs = work.tile([128, 512], F8, tag="probs", name="probs",
                                  bufs=4)
                with nc.allow_low_precision(reason="fp8 probs"):
                    nc.scalar.activation(probs, ps, AF.Exp, bias=eshift)
                pend.append((probs, b, hd, p))
                if len(pend) > 2:
                    emit_ctx(pend.pop(0))
                yield
                if p == 12:
                    yield from tail(0)
            while pend:
                emit_ctx(pend.pop(0))
                yield
            yield from tail(1)

        def interleave(dg, ag):
            # dg yields fill hints; -1 opens the gate (QKV of this stage is
            # fully emitted, so A steps cannot head-of-line block the PE).
            a_done = False
            gated = True

            def fill(n):
                nonlocal a_done
                for _ in range(n):
                    if a_done:
                        return
                    try:
                        next(ag)
                    except StopIteration:
                        a_done = True
                        return

            since_fill = 0
            for hint in dg:
                if hint == -1:
                    gated = False
                    continue
                if gated:
                    continue
                if hint:
                    fill(hint)
                    since_fill = 0
                else:
                    since_fill += 1
                    if since_fill >= 3:
                        fill(1)
                        since_fill = 0
            while not a_done:
                fill(1)

        # ---- prologue ----
        load_qkv(0)
        load_qkv(1)
        load_ffn(0)
        for _ in emit_ln1(0):
            pass

        # ---- main wave ----
        for i in range(S):
            if i % 2 == 0 and i // 2 + 1 < L:
                load_qkv(i // 2 + 1)
            if i % 2 == 1 and (i + 1) // 2 < L:
                load_ffn((i + 1) // 2)
            interleave(emit_D(i - 1), emit_A(i))

        # ---- epilogue: dense block of the last stage ----
        for _ in emit_D(S - 1):
            pass
        nc.sync.dma_start(y_out, xT)

    _legalize_sync(nc)
    return nc


_NC_CACHE = {}


def _get_nc():
    if "nc" not in _NC_CACHE:
        _NC_CACHE["nc"] = _build_nc()
    return _NC_CACHE["nc"]


def _prep_inputs(x, attn_bias, ln1_s, ln1_b, wq, bq, wk, bk, wv, bv, wo, bo,
                 ln2_s, ln2_b, w1, b1, w2, b2):
    f32 = np.float32
    asf = lambda a: np.asarray(a, dtype=f32)
    x, attn_bias = asf(x), asf(attn_bias)
    ln1_s, ln1_b, ln2_s, ln2_b = asf(ln1_s), asf(ln1_b), asf(ln2_s), asf(ln2_b)
    wq, wk, wv, wo, w1, w2 = asf(wq), asf(wk), asf(wv), asf(wo), asf(w1), asf(w2)
    bq, bk, bv, bo, b1, b2 = asf(bq), asf(bk), asf(bv), asf(bo), asf(b1), asf(b2)

    scale = f32(DH ** -0.5)
    # fold LN affine into the following matmuls; fold q-scale into wq/bq;
    # fold bv through wo into bo.
    wq_f = ln1_s[:, :, None] * wq * scale
    bq_f = (bq + np.einsum("lh,lho->lo", ln1_b, wq)) * scale
    wk_f = ln1_s[:, :, None] * wk
    bk_f = bk + np.einsum("lh,lho->lo", ln1_b, wk)
    wv_f = ln1_s[:, :, None] * wv
    bv_f = bv + np.einsum("lh,lho->lo", ln1_b, wv)
    bo_f = bo + np.einsum("lh,lho->lo", bv_f, wo)
    w1_f = ln2_s[:, :, None] * w1
    b1_f = b1 + np.einsum("lh,lhf->lf", ln2_b, w1)

    def q8(a):
        return np.asarray(np.clip(a, -240.0, 240.0), E4M3)

    def dr_qk_layout(w):  # [L, 512, 512] -> [L, 128, (i2, c2, oc4, 128)] fp8
        a = w.reshape(L, 2, 2, 128, 4, 128)        # (c, i, p, oc, o)
        a = a.transpose(0, 3, 2, 1, 4, 5)          # (p, i, c, oc, o)
        return np.ascontiguousarray(q8(a.reshape(L, 128, 2048)))

    def dr_v_layout(w):  # [L, 512, 512] -> [L, 128, (i2, c2, 512)] fp8
        a = w.reshape(L, 2, 2, 128, 512).transpose(0, 3, 2, 1, 4)
        return np.ascontiguousarray(q8(a.reshape(L, 128, 2048)))

    def lhsT_layout(w, ncon, nout):  # bf16 stationary (FFN)
        a = w.reshape(L, ncon, 128, nout, 128).transpose(0, 2, 1, 3, 4)
        return np.ascontiguousarray(a.reshape(L, 128, ncon * nout * 128)).astype(BF16)

    def b_layout(bvec, nch):
        return np.ascontiguousarray(
            bvec.reshape(L, nch, 128).transpose(0, 2, 1)).astype(f32)

    shared = {
        "wq_in": dr_qk_layout(wq_f),
        "wk_in": dr_qk_layout(wk_f),
        "wv_in": dr_v_layout(wv_f),
        "wo_in": dr_qk_layout(wo),
        "w1_in": lhsT_layout(w1_f, 4, 16),
        "w2_in": lhsT_layout(w2, 16, 4),
        "bq_in": b_layout(bq_f, 4),
        "bk_in": b_layout(bk_f, 4),
        "b1_in": b_layout(b1_f, 16),
        "bo_in": b_layout(bo_f, 4),
        "b2_in": b_layout(b2, 4),
        "id_in": q8(np.eye(128, dtype=f32)),
    }
    sel = np.zeros((64, 8, 128), dtype=f32)
    for p in range(8):
        b_, hcq = p // 4, p % 4
        j0 = b_ * 32 + 2 * hcq
        sel[j0, p, 0:64] = 1.0
        sel[j0 + 1, p, 64:128] = 1.0
    shared["sel_in"] = sel.astype(BF16)

    xs = x.reshape(NC, BL, N, H)
    ab = attn_bias.reshape(NC, BL, HEADS, N, N)
    in_maps = []
    for c in range(NC):
        xT_c = np.ascontiguousarray(
            xs[c].transpose(2, 0, 1).reshape(4, 128, T).transpose(1, 0, 2))
        # [b, h, q, k] -> [b, h, k, q] -> [b, h, ktc, p, q] -> [p, b*h, ktc*q]
        eb_c = ab[c].transpose(0, 1, 3, 2).reshape(BL, HEADS, 2, 128, 256)
        eb_c = np.ascontiguousarray(
            eb_c.transpose(3, 0, 1, 2, 4).reshape(128, BL * HEADS, 512))
        m = {"x_in": xT_c.reshape(128, 4, T), "eb_in": q8(eb_c)}
        m.update(shared)
        in_maps.append(m)
    return in_maps


def _run(inputs, trace=False):
    nc = _get_nc()
    in_maps = _prep_inputs(**inputs)
    res = run_bass_kernel_spmd(nc, in_maps, core_ids=list(range(NC)), trace=trace)
    outs = []
    for c in range(NC):
        yT = np.asarray(res.results[c]["y_out"], dtype=np.float32)
        y = yT.reshape(128, 4, T).transpose(1, 0, 2).reshape(H, BL, N).transpose(1, 2, 0)
        outs.append(y)
    full = np.ascontiguousarray(np.concatenate(outs, axis=0), dtype=np.float32)
    return full, res


def kernel(**inputs):
    full, _ = _run(inputs, trace=False)
    return full
